# revision 1
# baseline (speedup 1.0000x reference)
"""Causal self-attention Trainium2 Bass kernel.

Problem: B=4, T=2048, C=2048, H=16 heads, D=128 head dim, fp32.
  qkv = x @ w_qkv ; causal softmax(q k^T / sqrt(D)) v ; out = av @ w_proj

Sharding (8 NeuronCores): DP=4 over batch x TP=2 over head groups
(Megatron-style: w_qkv columns / w_proj rows split by heads). Core
c handles batch b=c//2, heads g*8..g*8+8 with g=c%2. Each core emits a
partial [T, C] output; host sums the TP pairs.

Per-core dataflow (transpose-free):
  Phase 1a: qT/kT = (x @ Wqk)^T computed directly in [d, t] layout:
            stationary = Wqk tile [c,128j], moving = xT [c, t512].
  Phase 1b: v in natural [t, d] layout: stationary = xT tile [c, t128],
            moving = Wv [c, 512].
  Phase 2 (per head): scoresT[s,t] = kT.T@qT blocks -> additive causal
            mask -> exp (ACT, scale=1/sqrt(D), no max-subtraction needed
            for N(0,1) logits) -> softmax denominators via ones-matmul
            (partition-dim sum) -> avT[d,t] += v_chunk.T @ expT.
            Normalization by 1/sum fused into the PSUM->SBUF copy.
  Phase 3: outT partial = avT.T @ w_proj rows, accumulated over the 8
            local heads' feature chunks.

All matmuls run as float32r (full PE rate at moving dim >= 256).
"""

import math
import os
import sys

import numpy as np

for _p in ("/opt/trn_rl_repo",):
    if _p not in sys.path:
        sys.path.insert(0, _p)

import concourse.bass as bass
import concourse.mybir as mybir
from concourse import bacc
from concourse.tile import TileContext

B, T, C, H, D = 4, 2048, 2048, 16, 128
P = 128
NCORES = 8
HL = 8          # heads per core (local)
FL = HL * D     # local feature dim = 1024
NCC = C // P    # 16 contraction chunks
NTB = T // 512  # 4 t superblocks
NTC = T // P    # 16 t chunks
EXP_SCALE = 1.0 / math.sqrt(D)
NEG = -1.0e30

f32 = mybir.dt.float32
f32r = mybir.dt.float32r


def _r(ap):
    return ap.bitcast(f32r)


def build_nc():
    nc = bacc.Bacc()
    xt_d = nc.declare_dram_parameter("xt", [C, T], f32r, isOutput=False)
    wqk_d = nc.declare_dram_parameter("wqk", [16, 16, P, P], f32r, isOutput=False)
    wv_d = nc.declare_dram_parameter("wv", [C, FL], f32r, isOutput=False)
    wp_d = nc.declare_dram_parameter("wp", [FL, C], f32r, isOutput=False)
    masks_d = nc.declare_dram_parameter("masks", [P, P], f32, isOutput=False)
    ones_d = nc.declare_dram_parameter("ones", [P, 1], f32r, isOutput=False)
    out_d = nc.declare_dram_parameter("out", [T, C], f32, isOutput=True)

    qk_scr = nc.dram_tensor("qk_scr", [16, P, T], f32r)
    v_scr = nc.dram_tensor("v_scr", [NTC, P, FL], f32r)

    ACT = mybir.ActivationFunctionType

    with TileContext(nc) as tc:
        with tc.tile_pool(name="const", bufs=1) as const_pool:
            ones = const_pool.tile([P, 1], f32r)
            nc.sync.dma_start(ones[:], ones_d[:])
            mask_sb = const_pool.tile([P, P], f32)
            nc.sync.dma_start(mask_sb[:], masks_d[:])

            # ---------------- Phase 1: QKV projection ----------------
            with tc.tile_pool(name="xtp", bufs=1) as xt_pool, \
                 tc.tile_pool(name="ps1", bufs=8, space="PSUM") as ps1:
                xts = []
                for cc in range(NCC):
                    t_ = xt_pool.tile([P, T], f32r, tag=f"xt{cc}")
                    xts.append(t_)

                # 1a: qT (j=0..7) and kT (j=8..15), [d,t] layout
                with tc.tile_pool(name="w1", bufs=48) as w_pool, \
                     tc.tile_pool(name="cb1", bufs=6) as cb1:
                    # jc 0-1 are emitted cc-outer so the PE's in-order queue
                    # tracks the xt chunk arrivals instead of head-of-line
                    # blocking on chunk 8 while the input DMA is in flight.
                    nc.sync.dma_start(xts[0][:], xt_d[0:P, :])
                    wts01 = [None] * (2 * NCC)
                    for cc in range(NCC):
                        for jc in range(2):
                            wt_ = w_pool.tile([P, P], f32r, tag="wqk",
                                              name=f"w01_{jc}_{cc}")
                            nc.sync.dma_start(wt_[:], wqk_d[jc, cc])
                            wts01[jc * NCC + cc] = wt_
                    for cc in range(1, NCC):
                        nc.sync.dma_start(xts[cc][:], xt_d[cc * P:(cc + 1) * P, :])
                    ps01 = [ps1.tile([P, 512], f32, tag="ps1",
                                     name=f"ps01_{g}") for g in range(8)]
                    for cc in range(NCC):
                        for jc in range(2):
                            for tb in range(NTB):
                                nc.tensor.matmul(
                                    ps01[jc * NTB + tb][:],
                                    _r(wts01[jc * NCC + cc][:]),
                                    _r(xts[cc][:, tb * 512:(tb + 1) * 512]),
                                    start=(cc == 0), stop=(cc == NCC - 1))
                    for jc in range(2):
                        for tb in range(NTB):
                            ob = cb1.tile([P, 512], f32r, tag="qkcb")
                            nc.vector.tensor_copy(
                                out=ob[:], in_=ps01[jc * NTB + tb][:])
                            nc.sync.dma_start(
                                qk_scr[jc, :, tb * 512:(tb + 1) * 512], ob[:])
                    for jc in range(2, 16):
                        wts = []
                        for cc in range(NCC):
                            wt_ = w_pool.tile([P, P], f32r, tag="wqk")
                            nc.sync.dma_start(wt_[:], wqk_d[jc, cc])
                            wts.append(wt_)
                        for tb in range(NTB):
                            ps = ps1.tile([P, 512], f32, tag="ps1")
                            for cc in range(NCC):
                                nc.tensor.matmul(
                                    ps[:], _r(wts[cc][:]),
                                    _r(xts[cc][:, tb * 512:(tb + 1) * 512]),
                                    start=(cc == 0), stop=(cc == NCC - 1))
                            ob = cb1.tile([P, 512], f32r, tag="qkcb")
                            nc.vector.tensor_copy(out=ob[:], in_=ps[:])
                            nc.sync.dma_start(
                                qk_scr[jc, :, tb * 512:(tb + 1) * 512], ob[:])

                # 1b: v in [t, d] layout
                with tc.tile_pool(name="wv1", bufs=16) as wv_pool, \
                     tc.tile_pool(name="cb2", bufs=6) as cb2:
                    for vb in range(FL // 512):
                        wvts = []
                        for cc in range(NCC):
                            wt_ = wv_pool.tile([P, 512], f32r, tag="wv")
                            nc.sync.dma_start(
                                wt_[:],
                                wv_d[cc * P:(cc + 1) * P, vb * 512:(vb + 1) * 512])
                            wvts.append(wt_)
                        for tb in range(NTC):
                            ps = ps1.tile([P, 512], f32, tag="ps1")
                            for cc in range(NCC):
                                nc.tensor.matmul(
                                    ps[:], _r(xts[cc][:, tb * P:(tb + 1) * P]),
                                    _r(wvts[cc][:]),
                                    start=(cc == 0), stop=(cc == NCC - 1))
                            ob = cb2.tile([P, 512], f32r, tag="vcb")
                            nc.vector.tensor_copy(out=ob[:], in_=ps[:])
                            nc.sync.dma_start(
                                v_scr[tb, :, vb * 512:(vb + 1) * 512], ob[:])

            # ---------------- Phase 2: attention per local head ----------------
            with tc.tile_pool(name="avt", bufs=1) as avt_pool:
                avts = [
                    avt_pool.tile([P, T], f32r, tag=f"avt{h}", name=f"avt{h}")
                    for h in range(HL)
                ]
                with tc.tile_pool(name="qkv2", bufs=3) as qkv_pool, \
                     tc.tile_pool(name="expp", bufs=8) as exp_pool, \
                     tc.tile_pool(name="st2", bufs=4) as st2, \
                     tc.tile_pool(name="ps2", bufs=1, space="PSUM") as ps2:
                    for h in range(HL):
                        qt = qkv_pool.tile([P, T], f32r, tag="qt")
                        nc.sync.dma_start(qt[:], qk_scr[h])
                        kt = qkv_pool.tile([P, T], f32r, tag="kt")
                        nc.sync.dma_start(kt[:], qk_scr[HL + h])
                        vt = qkv_pool.tile([P, NTC, P], f32r, tag="vt")
                        nc.gpsimd.dma_start(
                            vt[:],
                            v_scr[:, :, h * P:(h + 1) * P].rearrange("n p f -> p n f"))
                        for si in range(NTB):
                            av_ps = ps2.tile([P, 512], f32, tag="av", bufs=2)
                            sum_ps = ps2.tile([1, 512], f32, tag="sum", bufs=2)
                            njc = 4 * si + 4
                            for j in range(njc):
                                diag = j * P - si * 512
                                d_off = max(0, diag)
                                w = 512 - d_off
                                sc_ps = ps2.tile([P, 512], f32, tag="sc", bufs=4)
                                nc.tensor.matmul(
                                    sc_ps[:, :w], _r(kt[:, j * P:(j + 1) * P]),
                                    _r(qt[:, si * 512 + d_off:(si + 1) * 512]),
                                    start=True, stop=True)
                                if diag >= 0:
                                    nc.vector.tensor_add(
                                        out=sc_ps[:, :P], in0=sc_ps[:, :P],
                                        in1=mask_sb[:])
                                et = exp_pool.tile([P, 512], f32r, tag="exp")
                                nc.scalar.activation(
                                    et[:, :w], sc_ps[:, :w], ACT.Exp,
                                    scale=EXP_SCALE)
                                nc.tensor.matmul(
                                    sum_ps[:, d_off:], _r(ones[:]), _r(et[:, :w]),
                                    start=(j == 0), stop=(j == njc - 1))
                                nc.tensor.matmul(
                                    av_ps[:, d_off:], _r(vt[:, j, :]), _r(et[:, :w]),
                                    start=(j == 0), stop=(j == njc - 1))
                            rec = st2.tile([1, 512], f32, tag="rec")
                            nc.vector.reciprocal_approx_fast(
                                out=rec[:], in_=sum_ps[:])
                            recb = st2.tile([P, 512], f32, tag="recb")
                            nc.gpsimd.partition_broadcast(recb[:], rec[:])
                            nc.vector.tensor_mul(
                                out=avts[h][:, si * 512:(si + 1) * 512],
                                in0=av_ps[:], in1=recb[:])

                # ---------------- Phase 3: output projection ----------------
                with tc.tile_pool(name="wpp", bufs=1) as wp_pool, \
                     tc.tile_pool(name="cb3", bufs=6) as cb3, \
                     tc.tile_pool(name="ps3", bufs=8, space="PSUM") as ps3:
                    wps = []
                    for f in range(HL):
                        t_ = wp_pool.tile([P, C], f32r, tag=f"wp{f}")
                        eng = nc.sync if f % 2 == 0 else nc.gpsimd
                        eng.dma_start(t_[:], wp_d[f * P:(f + 1) * P, :])
                        wps.append(t_)
                    for tch in range(NTC):
                        for cb_ in range(C // 512):
                            ps = ps3.tile([P, 512], f32, tag="ps3")
                            for f in range(HL):
                                nc.tensor.matmul(
                                    ps[:], _r(avts[f][:, tch * P:(tch + 1) * P]),
                                    _r(wps[f][:, cb_ * 512:(cb_ + 1) * 512]),
                                    start=(f == 0), stop=(f == HL - 1))
                            ob = cb3.tile([P, 512], f32, tag="ocb")
                            nc.any.tensor_copy(out=ob[:], in_=ps[:])
                            nc.sync.dma_start(
                                out_d[tch * P:(tch + 1) * P,
                                      cb_ * 512:(cb_ + 1) * 512], ob[:])
    nc.compile()
    return nc


def _make_masks():
    pp = np.arange(P)[:, None]
    ff = np.arange(P)[None, :]
    return np.where(ff >= pp, 0.0, NEG).astype(np.float32)


def _prep_inputs(x, w_qkv, w_proj):
    masks = _make_masks()
    per_g = {}
    for g in range(2):
        cols = slice(g * FL, (g + 1) * FL)
        wqk_c = np.concatenate(
            [w_qkv[:, cols], w_qkv[:, C:][:, cols]], axis=1)  # [C, 2048]
        wqk_packed = np.ascontiguousarray(
            wqk_c.reshape(NCC, P, 16, P).transpose(2, 0, 1, 3))
        wv_c = np.ascontiguousarray(w_qkv[:, 2 * C:][:, cols])
        wp_c = np.ascontiguousarray(w_proj[g * FL:(g + 1) * FL, :])
        per_g[g] = (wqk_packed, wv_c, wp_c)
    in_maps = []
    for core in range(NCORES):
        b, g = core // 2, core % 2
        wqk_packed, wv_c, wp_c = per_g[g]
        in_maps.append({
            "xt": np.ascontiguousarray(x[b].T),
            "wqk": wqk_packed,
            "wv": wv_c,
            "wp": wp_c,
            "masks": masks,
            "ones": np.ones((P, 1), dtype=np.float32),
        })
    return in_maps


_nc_cache = None
last_results = None  # BassKernelResults of the most recent run (for test.py)


def kernel(x, w_qkv, w_proj):
    global _nc_cache, last_results
    from concourse.bass_utils import run_bass_kernel_spmd

    x = np.asarray(x, dtype=np.float32)
    w_qkv = np.asarray(w_qkv, dtype=np.float32)
    w_proj = np.asarray(w_proj, dtype=np.float32)

    if _nc_cache is None:
        _nc_cache = build_nc()
    nc = _nc_cache

    in_maps = _prep_inputs(x, w_qkv, w_proj)
    trace = bool(int(os.environ.get("KERNEL_TRACE", "0")))
    res = run_bass_kernel_spmd(nc, in_maps, list(range(NCORES)), trace=trace)
    last_results = res

    out = np.empty((B, T, C), dtype=np.float32)
    for b in range(B):
        out[b] = res.results[2 * b]["out"] + res.results[2 * b + 1]["out"]
    return out

